# revision 1
# baseline (speedup 1.0000x reference)
"""Trainium kernel for nn_Interaction_1537598292236 (sparse_attention).

Strategy: data-parallel over batch N=8 across the 8 NeuronCores (every op in
the module is batch-independent: per-sample attention, per-row topk
thresholds, layernorm over features).  The full forward for one sample is
compiled onto one core via the XLA->neuronx-cc path and dispatched with
jax.pmap; inputs are sharded on the batch axis and the output gathered back
to the full [2, 8, 1024, 512] shape.

The reference's torch-faithful `reshape(-1, B, L, dh)` head split mixes the
(L, dk) axes but never mixes batch elements: flat chunk c = b*4 + l//256.
Restricted to a single sample (B=1) the reshape yields exactly that sample's
4 chunks, so the per-sample forward below is bit-identical in structure to
the batched reference.
"""

import math

import numpy as np

B, L, D_MODEL, N_HEADS = 8, 1024, 512, 4
DIM_K = D_MODEL // N_HEADS
HIDDEN = 4 * D_MODEL
SCALE = 1.0 / math.sqrt(D_MODEL)
EPS = 1e-5
NEG = -1e30


def _build_forward(jnp):
    def _lin(x, w, b):
        return x @ w + b

    def _layer_norm(x, g, b):
        mu = x.mean(-1, keepdims=True)
        var = ((x - mu) ** 2).mean(-1, keepdims=True)
        return (x - mu) / jnp.sqrt(var + EPS) * g + b

    def _softmax_masked(s, keep):
        # softmax over last axis with `keep` mask; masked lanes get a large
        # negative score (exp -> 0).  Max-subtraction keeps exp in range on HW.
        s = jnp.where(keep, s, NEG)
        m = s.max(-1, keepdims=True)
        e = jnp.exp(s - m)
        return e / e.sum(-1, keepdims=True)

    def _mha(x, y, p):
        # x, y: [L, D].  Per-sample version of the reference (Bb == 1).
        dim_k = p['wq'].shape[1]
        dh = dim_k // N_HEADS
        Q = _lin(x, p['wq'], p['bq']).reshape(-1, L, dh)   # [4, 1024, 32]
        K = _lin(y, p['wk'], p['bk']).reshape(-1, L, dh)
        V = _lin(y, p['wv'], p['bv']).reshape(-1, L, dh)
        s = jnp.einsum('hqd,hkd->hqk', Q, K) * SCALE
        keep = s > s.mean(-1, keepdims=True)
        a = _softmax_masked(s, keep)
        o = jnp.einsum('hqk,hkd->hqd', a, V).reshape(L, -1)
        return _lin(o, p['wo'], p['bo'])

    def _ffn(x, p):
        return _lin(jnp.maximum(_lin(x, p['w1'], p['b1']), 0.0), p['w2'], p['b2'])

    def _decoder(q, v, p):
        h = _layer_norm(q + _mha(q, v, p['mha']), p['ln_g'], p['ln_b'])
        h = _layer_norm(h + _ffn(h, p['ffn']), p['ln_g'], p['ln_b'])
        return h + q

    def _col_score(a, eye_off):
        # a: [L, D]; softmax over rows of a@a.T, zero diagonal, column sums.
        s = (a @ a.T) * SCALE
        m = s.max(-1, keepdims=True)
        e = jnp.exp(s - m)
        att = e / e.sum(-1, keepdims=True)
        att = att * eye_off
        return att.sum(axis=0)                             # [L]

    def _median_mask(s):
        # mask = s <= (k-th smallest of s), k = L//2, via rank counting
        # (avoids device sort).  Ties match torch/jnp.sort semantics: every
        # element whose strictly-less count is < k is kept.
        cnt = (s[None, :] < s[:, None]).astype(s.dtype).sum(-1)   # [L]
        return cnt < (L // 2)

    def _tokenchange(x_t, x_f, proj_w, proj_b, eye_off):
        xt = _lin(x_t, proj_w, proj_b)
        xf = _lin(x_f, proj_w, proj_b)
        s_t = _col_score(xt, eye_off)
        s_f = _col_score(xf, eye_off)
        mask_t = _median_mask(s_t)
        mask_f = _median_mask(s_f)
        mask_t1 = (mask_t & ~mask_f)[:, None]
        mask_f1 = (mask_f & ~mask_t)[:, None]
        x_t1 = jnp.where(mask_t1, x_f, x_t)
        x_f1 = jnp.where(mask_f1, x_t, x_f)
        return 0.5 * (x_t1 + x_t), 0.5 * (x_f1 + x_f)

    def forward_one(x, params):
        # x: [2, L, D] -- one batch sample of both streams.
        x_t, x_f = x[0], x[1]
        eye_off = 1.0 - jnp.eye(L, dtype=x.dtype)
        x_t2 = _decoder(x_t, x_f, params['t'])
        x_f2 = _decoder(x_f, x_t, params['f'])
        x_t2, x_f2 = _tokenchange(
            x_t2, x_f2, params['proj_w'], params['proj_b'], eye_off)
        x_t2 = _layer_norm(x_t2 + _mha(x_t2, x_t2, params['at']),
                           params['lnt_g'], params['lnt_b'])
        x_f2 = _layer_norm(x_f2 + _mha(x_f2, x_f2, params['af']),
                           params['lnf_g'], params['lnf_b'])
        return jnp.stack([x_t2 + x_t, x_f2 + x_f])         # [2, L, D]

    return forward_one


_CACHE = {}


def _get_pmapped():
    if 'f' in _CACHE:
        return _CACHE['f']
    import jax
    import jax.numpy as jnp

    devs = [d for d in jax.devices() if d.platform != 'cpu'][:8]
    if not devs:
        devs = jax.devices()[:8]
    forward_one = _build_forward(jnp)
    # batch axis 0 sharded over the 8 cores; params broadcast to every core.
    f = jax.pmap(forward_one, in_axes=(0, None), devices=devs)
    _CACHE['f'] = f
    return f


def kernel(x, params):
    x = np.asarray(x, dtype=np.float32)
    f = _get_pmapped()
    # reshape [2, B, L, D] -> [B, 2, L, D] so the batch axis leads for pmap
    xb = np.ascontiguousarray(np.transpose(x, (1, 0, 2, 3)))
    out = f(xb, params)                                    # [B, 2, L, D]
    out = np.asarray(out)
    return np.ascontiguousarray(np.transpose(out, (1, 0, 2, 3)))


# revision 2
# speedup vs baseline: 3.4269x; 3.4269x over previous
"""Trainium kernel for nn_Interaction_1537598292236 (sparse_attention).

Strategy: data-parallel over batch N=8 across the 8 NeuronCores (every op in
the module is batch-independent: per-sample attention, per-row topk
thresholds, layernorm over features).  The full forward for one sample is
compiled onto one core via the XLA->neuronx-cc path and dispatched with
jax.pmap; inputs are sharded on the batch axis and the output gathered back
to the full [2, 8, 1024, 512] shape.

The reference's torch-faithful `reshape(-1, B, L, dh)` head split mixes the
(L, dk) axes but never mixes batch elements: flat chunk c = b*4 + l//256.
Restricted to a single sample (B=1) the reshape yields exactly that sample's
4 chunks, so the per-sample forward below is bit-identical in structure to
the batched reference.
"""

import math

import numpy as np

B, L, D_MODEL, N_HEADS = 8, 1024, 512, 4
DIM_K = D_MODEL // N_HEADS
HIDDEN = 4 * D_MODEL
SCALE = 1.0 / math.sqrt(D_MODEL)
EPS = 1e-5
NEG = -1e30


def _build_forward(jnp):
    def _lin(x, w, b):
        return x @ w + b

    def _layer_norm(x, g, b):
        mu = x.mean(-1, keepdims=True)
        var = ((x - mu) ** 2).mean(-1, keepdims=True)
        return (x - mu) / jnp.sqrt(var + EPS) * g + b

    def _softmax_masked(s, keep):
        # softmax over last axis with `keep` mask; masked lanes get a large
        # negative score (exp -> 0).  Max-subtraction keeps exp in range on HW.
        s = jnp.where(keep, s, NEG)
        m = s.max(-1, keepdims=True)
        e = jnp.exp(s - m)
        return e / e.sum(-1, keepdims=True)

    def _mha(x, y, p):
        # x, y: [L, D].  Per-sample version of the reference (Bb == 1).
        dim_k = p['wq'].shape[1]
        dh = dim_k // N_HEADS
        Q = _lin(x, p['wq'], p['bq']).reshape(-1, L, dh)   # [4, 1024, 32]
        K = _lin(y, p['wk'], p['bk']).reshape(-1, L, dh)
        V = _lin(y, p['wv'], p['bv']).reshape(-1, L, dh)
        s = jnp.einsum('hqd,hkd->hqk', Q, K) * SCALE
        keep = s > s.mean(-1, keepdims=True)
        a = _softmax_masked(s, keep)
        o = jnp.einsum('hqk,hkd->hqd', a, V).reshape(L, -1)
        return _lin(o, p['wo'], p['bo'])

    def _ffn(x, p):
        return _lin(jnp.maximum(_lin(x, p['w1'], p['b1']), 0.0), p['w2'], p['b2'])

    def _decoder(q, v, p):
        h = _layer_norm(q + _mha(q, v, p['mha']), p['ln_g'], p['ln_b'])
        h = _layer_norm(h + _ffn(h, p['ffn']), p['ln_g'], p['ln_b'])
        return h + q

    def _col_score(a, eye_off):
        # a: [L, D]; softmax over rows of a@a.T, zero diagonal, column sums.
        s = (a @ a.T) * SCALE
        m = s.max(-1, keepdims=True)
        e = jnp.exp(s - m)
        att = e / e.sum(-1, keepdims=True)
        att = att * eye_off
        return att.sum(axis=0)                             # [L]

    def _median_mask(s):
        # mask = s <= (k-th smallest of s), k = L//2, via rank counting
        # (avoids device sort).  Ties match torch/jnp.sort semantics: every
        # element whose strictly-less count is < k is kept.
        cnt = (s[None, :] < s[:, None]).astype(s.dtype).sum(-1)   # [L]
        return cnt < (L // 2)

    def _tokenchange(x_t, x_f, proj_w, proj_b, eye_off):
        xt = _lin(x_t, proj_w, proj_b)
        xf = _lin(x_f, proj_w, proj_b)
        s_t = _col_score(xt, eye_off)
        s_f = _col_score(xf, eye_off)
        mask_t = _median_mask(s_t)
        mask_f = _median_mask(s_f)
        mask_t1 = (mask_t & ~mask_f)[:, None]
        mask_f1 = (mask_f & ~mask_t)[:, None]
        x_t1 = jnp.where(mask_t1, x_f, x_t)
        x_f1 = jnp.where(mask_f1, x_t, x_f)
        return 0.5 * (x_t1 + x_t), 0.5 * (x_f1 + x_f)

    def forward_one(x, params):
        # x: [2, L, D] -- one batch sample of both streams.
        x_t, x_f = x[0], x[1]
        eye_off = 1.0 - jnp.eye(L, dtype=x.dtype)
        x_t2 = _decoder(x_t, x_f, params['t'])
        x_f2 = _decoder(x_f, x_t, params['f'])
        x_t2, x_f2 = _tokenchange(
            x_t2, x_f2, params['proj_w'], params['proj_b'], eye_off)
        x_t2 = _layer_norm(x_t2 + _mha(x_t2, x_t2, params['at']),
                           params['lnt_g'], params['lnt_b'])
        x_f2 = _layer_norm(x_f2 + _mha(x_f2, x_f2, params['af']),
                           params['lnf_g'], params['lnf_b'])
        return jnp.stack([x_t2 + x_t, x_f2 + x_f])         # [2, L, D]

    return forward_one


_CACHE = {}


def _get_pmapped():
    if 'f' in _CACHE:
        return _CACHE['f'], _CACHE['devs']
    import jax
    import jax.numpy as jnp

    devs = [d for d in jax.devices() if d.platform != 'cpu'][:8]
    if not devs:
        devs = jax.devices()[:8]
    forward_one = _build_forward(jnp)
    # batch axis 0 sharded over the 8 cores; params replicated on every core.
    f = jax.pmap(forward_one, in_axes=(0, 0), devices=devs)
    _CACHE['f'] = f
    _CACHE['devs'] = devs
    return f, devs


def _device_params(params, devs):
    # Replicate the (call-invariant) weights onto all cores once; reuse on
    # subsequent calls so steady-state calls only move the activations.
    import jax
    key = id(params)
    if _CACHE.get('pkey') == key:
        return _CACHE['pdev']
    pdev = jax.device_put_replicated(params, devs)
    _CACHE['pkey'] = key
    _CACHE['pdev'] = pdev
    return pdev


def kernel(x, params):
    import jax

    x = np.asarray(x, dtype=np.float32)
    f, devs = _get_pmapped()
    pdev = _device_params(params, devs)
    # [2, B, L, D] -> per-sample shards [2, L, D] placed on each core
    xb = jax.device_put_sharded(
        [np.ascontiguousarray(x[:, b]) for b in range(B)], devs)
    out = f(xb, pdev)                                      # [B, 2, L, D]
    out = np.asarray(out)
    return np.ascontiguousarray(np.transpose(out, (1, 0, 2, 3)))
